# revision 25
# baseline (speedup 1.0000x reference)
"""Trainium2 Bass kernel for nn_AttentionLayer (method='general' attention).

Reference computation:
    proj[l,b,:] = W @ enc[l,b,:] + bias          # [L,B,H]
    e[b,l]      = hidden[0,b,:] . proj[l,b,:]    # [B,L]
    attn        = softmax(e, axis=0 over b)[:, None, :]   # [B,1,L]

Algebraic rewrite (exact up to rounding):
    u[b,:] = hidden[0,b,:] @ W      (64x1024, tiny)
    c[b]   = hidden[0,b,:] . bias
    e[l,b] = u[b,:] . enc[l,b,:] + c[b]
which turns a 275-GFLOP matmul into a streaming dot-product problem.

v2 (this file): the stream is HBM-bandwidth-bound, so enc ships as fp16
(256MB instead of 512MB; measured end-to-end rel err 1.9e-3 vs the 2e-2
gate). At fp16 stream rates the DVE (no 2x mode for scalar_tensor_tensor)
can no longer keep up, so the dot products move to the otherwise-idle PE:

  - Host pre-transposes each core's enc shard to h-major per batch:
    tile[p, hb*512 + b2*256 + j] = enc[l0+j, 2bp+b2, hb*128+p], one
    [128, 4096] fp16 tile per pair of batches (8KB/partition DMAs).
  - uT[hb] = (h @ W) slice as [128 h, 64 b] fp16 stationaries (computed
    on-chip from fp16 W row-tiles and host-pretransposed hT).
  - Per b-pair: one 8-matmul PSUM chain over hb: out[64, 512] where
    row b' holds u[b'].enc[l, b]; only rows 2bp/2bp+1 are real energies.
  - Extraction: two [1, 256] PSUM->SBUF copies per chain (ACT + DVE)
    build e_full[64 b, 256 l]; add c[b]; two PE transposes put b on the
    free axis; softmax over each 64-wide row half; out [256, 64] fp32.

Sharding: L axis (2048) split across 8 cores (256 rows each). The softmax
is over the batch axis, which stays fully local per core, so no collectives.
"""

import numpy as np

L_FULL, B, H = 2048, 64, 1024
N_CORES = 8
L_SHARD = L_FULL // N_CORES          # 256
NBP = B // 2                         # 32 batch-pairs -> 32 enc DMA tiles/core

_PROGRAM = None


def _build_program():
    import concourse.bacc as bacc
    import concourse.mybir as mybir
    from concourse import masks, tile
    from concourse.tile import add_dep_helper

    f32 = mybir.dt.float32
    f16 = mybir.dt.float16
    bf16 = mybir.dt.bfloat16
    nc = bacc.Bacc(None)

    enc_in = nc.declare_dram_parameter("enc", [NBP, 128, 4096], f16, isOutput=False)
    ht_in = nc.declare_dram_parameter("ht", [128, 512], f16, isOutput=False)
    w_in = nc.declare_dram_parameter("w", [8, 128, H], f16, isOutput=False)
    bv_in = nc.declare_dram_parameter("bv", [128, 8], f16, isOutput=False)
    out_t = nc.declare_dram_parameter("attn", [L_SHARD, B], f32, isOutput=True)

    with tile.TileContext(nc) as tc:
        with (
            tc.tile_pool(name="const", bufs=1) as constp,
            tc.tile_pool(name="wpool", bufs=1) as wpool,
            tc.tile_pool(name="encp", bufs=6) as encp,
            tc.tile_pool(name="small", bufs=4) as smallp,
            tc.tile_pool(name="psS", bufs=4, space="PSUM") as psS,
            tc.tile_pool(name="psX", bufs=4, space="PSUM") as psX,
        ):
            ident = constp.tile([128, 128], f32)
            masks.make_identity(nc, ident[:])

            # hT [128, 512]: hT[p, kb*64+b] = hidden[b, kb*128+p] (host-built)
            ht_sb = constp.tile([128, 512], f16)
            pre_dmas = [nc.sync.dma_start(ht_sb[:], ht_in[:]).ins]
            # W row-tiles [128, 1024] fp16 x8, explicitly ordered before the
            # enc stream so the stationaries are ready early.
            w_tiles = []
            for q in range(8):
                wt_ = wpool.tile([128, H], f16, name=f"w{q}", tag=f"w{q}")
                pre_dmas.append(nc.sync.dma_start(wt_[:], w_in[q]).ins)
                w_tiles.append(wt_)
            bv_sb = constp.tile([128, 8], f16)
            nc.scalar.dma_start(bv_sb[:], bv_in[:])

            # uT[hb] [128 h, 64 b] fp16: uT[h, b] = sum_k W[k, h] h[b, k]
            uT = []
            for hb in range(8):
                ups = psX.tile([128, B], f32, name=f"ups{hb}", tag="psx")
                for kb in range(8):
                    nc.tensor.matmul(
                        ups[:],
                        w_tiles[kb][:, hb * 128 : (hb + 1) * 128],
                        ht_sb[:, kb * B : (kb + 1) * B],
                        start=(kb == 0),
                        stop=(kb == 7),
                        skip_group_check=True,
                    )
                t = constp.tile([128, B], f16, name=f"uT{hb}", tag=f"uT{hb}")
                nc.vector.tensor_copy(t[:], ups[:])
                uT.append(t)

            # c[b] = h[b] . bias -> c2 [64, 1] f32
            cpt = psX.tile([128, B], f32, name="cps", tag="psx")
            cps = cpt[0:B, 0:1]
            for kb in range(8):
                nc.tensor.matmul(
                    cps,
                    ht_sb[:, kb * B : (kb + 1) * B],
                    bv_sb[:, kb : kb + 1],
                    start=(kb == 0),
                    stop=(kb == 7),
                    skip_group_check=True,
                )
            c2 = smallp.tile([B, 1], f32)
            nc.scalar.copy(c2[:], cps)

            # Broadcast c[b] across partitions: transpose c2 -> cT [1, 64],
            # cast to fp16, outer-product with a ones column to get
            # cb[128, 64] (cb[p, b] = c[b]). Keeps every engine access at
            # partition base 0 (arbitrary partition bases fail birverifier).
            ctp = psX.tile([128, B], f32, name="ctp", tag="psx")
            nc.tensor.transpose(ctp[0:1, 0:B], c2[:], ident[:B, :B])
            cT16 = smallp.tile([1, B], f16)
            nc.scalar.copy(cT16[:], ctp[0:1, 0:B])
            ones16 = constp.tile([1, 128], f16)
            nc.vector.memset(ones16[:], 1.0)
            cbp = psX.tile([128, B], f32, name="cbp", tag="psx")
            nc.tensor.matmul(cbp[:], ones16[:], cT16[:], start=True, stop=True,
                             skip_group_check=True)
            cb = smallp.tile([128, B], f32)
            nc.scalar.copy(cb[:], cbp[:])

            # Main stream: one [128, 4096] fp16 tile per b-pair; 8-matmul
            # PSUM chain over hb produces ps[b', b2*256+j] = u[b'].enc[l0+j,
            # 2bp+b2]; only rows 2bp (first half) / 2bp+1 (second half) are
            # real energies. Extract via bulk PSUM->SBUF copy + four PE
            # transposes + single-column copies into eT tiles laid out
            # [l(128 part), b(64 free)] so the softmax needs no final
            # transpose.
            eT = [constp.tile([128, B], f32, name=f"eT{h}", tag=f"eT{h}")
                  for h in range(2)]

            def emit_transposes(s2, bpA, bpB):
                # 4 [128,128] PE transposes cover both stacked chains; copy
                # out the 2 useful columns each.
                for q in range(4):
                    half, b2 = q % 2, q // 2
                    pt = psX.tile([128, 128], f32, name="pt", tag="psx")
                    nc.tensor.transpose(
                        pt[:], s2[:, q * 128 : (q + 1) * 128], ident[:]
                    )
                    bA, bB = 2 * bpA + b2, 2 * bpB + b2
                    nc.scalar.copy(eT[half][:, bA : bA + 1], pt[:, bA : bA + 1])
                    nc.vector.tensor_copy(
                        eT[half][:, bB : bB + 1], pt[:, B + bB : B + bB + 1]
                    )

            # Software pipeline. Per group g: chains (PE), then the PSUM->
            # SBUF bulk copies for g IMMEDIATELY (so their semaphore waits
            # only cover g's chains — issuing them later coarsens the wait
            # to include the next group's matmuls), then group g-1's
            # transposes, which find s2(g-1) long since written.
            pend = None
            for g in range(NBP // 2):
                ts, chains = [], []
                for j in range(2):
                    bp = 2 * g + j
                    t = encp.tile([128, 4096], f16)
                    enc_dma = nc.sync.dma_start(t[:], enc_in[bp])
                    if bp < 4:
                        for w in pre_dmas:
                            add_dep_helper(
                                enc_dma.ins, w, sync=False,
                                reason="precompute DMAs drain before enc stream",
                            )
                    ts.append(t)
                    chains.append(psS.tile([B, 512], f32, name="ps", tag="ps"))
                for hb in range(8):
                    for j in range(2):
                        nc.tensor.matmul(
                            chains[j][:],
                            uT[hb][:],
                            ts[j][:, hb * 512 : (hb + 1) * 512],
                            start=(hb == 0),
                            stop=(hb == 7),
                            skip_group_check=True,
                        )
                # Stack both chains into s2 (chain A rows 0..63, B rows
                # 64..127); these run on ACT/DVE concurrent with the next
                # group's chains on PE.
                s2 = smallp.tile([128, 512], f32, name="s2", tag="s2", bufs=3)
                nc.scalar.copy(s2[0:B, :], chains[0][:])
                nc.vector.tensor_copy(s2[B:128, :], chains[1][:])
                if pend is not None:
                    emit_transposes(*pend)
                pend = (s2, 2 * g, 2 * g + 1)
            emit_transposes(*pend)

            for half in range(2):
                eTc = smallp.tile([128, B], f32)
                nc.vector.tensor_add(eTc[:], eT[half][:], cb[:])
                nm = smallp.tile([128, 1], f32)
                nc.vector.tensor_reduce(
                    nm[:],
                    eTc[:],
                    axis=mybir.AxisListType.X,
                    op=mybir.AluOpType.max,
                    negate=True,
                )
                ex = smallp.tile([128, B], f32)
                ssum = smallp.tile([128, 1], f32)
                nc.scalar.activation(
                    ex[:],
                    eTc[:],
                    mybir.ActivationFunctionType.Exp,
                    bias=nm[:, 0:1],
                    scale=1.0,
                    accum_out=ssum[:],
                )
                rec = smallp.tile([128, 1], f32)
                nc.vector.reciprocal(rec[:], ssum[:])
                attn_sb = smallp.tile([128, B], f32)
                nc.vector.tensor_scalar_mul(attn_sb[:], ex[:], rec[:, 0:1])
                nc.sync.dma_start(out_t[half * 128 : (half + 1) * 128, :], attn_sb[:])

    nc.finalize()
    return nc


def _get_program():
    global _PROGRAM
    if _PROGRAM is None:
        _PROGRAM = _build_program()
    return _PROGRAM


def _prep_inputs(inputs):
    """Build the 8 per-core input maps (all fp16 except nothing)."""
    f16 = np.float16
    hidden = np.asarray(inputs["hidden"], dtype=np.float32)
    enc = np.asarray(inputs["encoder_outputs"], dtype=np.float32)
    W = np.asarray(inputs["W"], dtype=np.float32)
    b = np.asarray(inputs["b"], dtype=np.float32)

    # enc[l, b, h] -> per core: tile[bp, p, hb*512 + b2*256 + j]
    #   = enc[core*256 + j, 2bp + b2, hb*128 + p]
    enc16 = np.ascontiguousarray(enc).astype(f16)
    E = enc16.reshape(N_CORES, L_SHARD, NBP, 2, 8, 128)  # [core, j, bp, b2, hb, p]
    P = np.ascontiguousarray(E.transpose(0, 2, 5, 4, 3, 1))  # [core, bp, p, hb, b2, j]
    P = P.reshape(N_CORES, NBP, 128, 4096)

    ht = np.ascontiguousarray(
        hidden[0].astype(f16).reshape(B, 8, 128).transpose(2, 1, 0).reshape(128, 512)
    )
    w = np.ascontiguousarray(W.astype(f16).reshape(8, 128, H))
    bv = np.ascontiguousarray(b.astype(f16).reshape(8, 128).T)

    return [
        {"enc": P[k], "ht": ht, "w": w, "bv": bv} for k in range(N_CORES)
    ]


def kernel(**inputs) -> np.ndarray:
    from concourse.bass_utils import run_bass_kernel_spmd

    nc = _get_program()
    in_maps = _prep_inputs(inputs)
    res = run_bass_kernel_spmd(nc, in_maps, list(range(N_CORES)))

    outs = []
    for k in range(N_CORES):
        a = np.asarray(res.results[k]["attn"])  # [L_SHARD, B]
        outs.append(a.T)                        # [B, L_SHARD]
    out = np.concatenate(outs, axis=1)[:, None, :].astype(np.float32)
    return out


# revision 28
# speedup vs baseline: 1.1912x; 1.1912x over previous
"""Trainium2 Bass kernel for nn_AttentionLayer (method='general' attention).

Reference computation:
    proj[l,b,:] = W @ enc[l,b,:] + bias          # [L,B,H]
    e[b,l]      = hidden[0,b,:] . proj[l,b,:]    # [B,L]
    attn        = softmax(e, axis=0 over b)[:, None, :]   # [B,1,L]

Algebraic rewrite (exact up to rounding):
    u[b,:] = hidden[0,b,:] @ W      (64x1024, tiny)
    c[b]   = hidden[0,b,:] . bias
    e[l,b] = u[b,:] . enc[l,b,:] + c[b]
which turns a 275-GFLOP matmul into a streaming dot-product problem.

v2 (this file): the stream is HBM-bandwidth-bound, so enc ships as fp16
(256MB instead of 512MB; measured end-to-end rel err 1.9e-3 vs the 2e-2
gate). At fp16 stream rates the DVE (no 2x mode for scalar_tensor_tensor)
can no longer keep up, so the dot products move to the otherwise-idle PE:

  - Host pre-transposes each core's enc shard to h-major per batch:
    tile[p, hb*512 + b2*256 + j] = enc[l0+j, 2bp+b2, hb*128+p], one
    [128, 4096] fp16 tile per pair of batches (8KB/partition DMAs).
  - uT[hb] = (h @ W) slice as [128 h, 64 b] fp16 stationaries (computed
    on-chip from fp16 W row-tiles and host-pretransposed hT).
  - Per b-pair: one 8-matmul PSUM chain over hb: out[64, 512] where
    row b' holds u[b'].enc[l, b]; only rows 2bp/2bp+1 are real energies.
  - Extraction: two [1, 256] PSUM->SBUF copies per chain (ACT + DVE)
    build e_full[64 b, 256 l]; add c[b]; two PE transposes put b on the
    free axis; softmax over each 64-wide row half; out [256, 64] fp32.

Sharding: L axis (2048) split across 8 cores (256 rows each). The softmax
is over the batch axis, which stays fully local per core, so no collectives.
"""

import numpy as np

L_FULL, B, H = 2048, 64, 1024
N_CORES = 8
L_SHARD = L_FULL // N_CORES          # 256
NBP = B // 2                         # 32 batch-pairs -> 32 enc DMA tiles/core

_PROGRAM = None


def _build_program():
    import concourse.bacc as bacc
    import concourse.mybir as mybir
    from concourse import masks, tile
    from concourse.tile import add_dep_helper

    f32 = mybir.dt.float32
    f16 = mybir.dt.float16
    bf16 = mybir.dt.bfloat16
    nc = bacc.Bacc(None)

    enc_in = nc.declare_dram_parameter("enc", [NBP, 128, 4096], f16, isOutput=False)
    ht_in = nc.declare_dram_parameter("ht", [128, 512], f16, isOutput=False)
    w_in = nc.declare_dram_parameter("w", [8, 128, H], f16, isOutput=False)
    bv_in = nc.declare_dram_parameter("bv", [128, 8], f16, isOutput=False)
    out_t = nc.declare_dram_parameter("attn", [L_SHARD, B], f32, isOutput=True)

    with tile.TileContext(nc) as tc:
        with (
            tc.tile_pool(name="const", bufs=1) as constp,
            tc.tile_pool(name="wpool", bufs=1) as wpool,
            tc.tile_pool(name="encp", bufs=6) as encp,
            tc.tile_pool(name="small", bufs=4) as smallp,
            tc.tile_pool(name="psS", bufs=4, space="PSUM") as psS,
            tc.tile_pool(name="psX", bufs=4, space="PSUM") as psX,
        ):
            ident = constp.tile([128, 128], f32)
            masks.make_identity(nc, ident[:])

            # hT [128, 512]: hT[p, kb*64+b] = hidden[b, kb*128+p] (host-built)
            ht_sb = constp.tile([128, 512], f16)
            pre_dmas = [nc.sync.dma_start(ht_sb[:], ht_in[:]).ins]
            # W row-tiles [128, 1024] fp16 x8, explicitly ordered before the
            # enc stream so the stationaries are ready early.
            w_tiles = []
            for q in range(8):
                wt_ = wpool.tile([128, H], f16, name=f"w{q}", tag=f"w{q}")
                pre_dmas.append(nc.sync.dma_start(wt_[:], w_in[q]).ins)
                w_tiles.append(wt_)
            bv_sb = constp.tile([128, 8], f16)
            nc.scalar.dma_start(bv_sb[:], bv_in[:])

            # uT[hb] [128 h, 64 b] fp16: uT[h, b] = sum_k W[k, h] h[b, k]
            uT = []
            for hb in range(8):
                ups = psX.tile([128, B], f32, name=f"ups{hb}", tag="psx")
                for kb in range(8):
                    nc.tensor.matmul(
                        ups[:],
                        w_tiles[kb][:, hb * 128 : (hb + 1) * 128],
                        ht_sb[:, kb * B : (kb + 1) * B],
                        start=(kb == 0),
                        stop=(kb == 7),
                        skip_group_check=True,
                    )
                t = constp.tile([128, B], f16, name=f"uT{hb}", tag=f"uT{hb}")
                nc.vector.tensor_copy(t[:], ups[:])
                uT.append(t)

            # c[b] = h[b] . bias -> c2 [64, 1] f32
            cpt = psX.tile([128, B], f32, name="cps", tag="psx")
            cps = cpt[0:B, 0:1]
            for kb in range(8):
                nc.tensor.matmul(
                    cps,
                    ht_sb[:, kb * B : (kb + 1) * B],
                    bv_sb[:, kb : kb + 1],
                    start=(kb == 0),
                    stop=(kb == 7),
                    skip_group_check=True,
                )
            c2 = smallp.tile([B, 1], f32)
            nc.scalar.copy(c2[:], cps)

            # Broadcast c[b] across partitions: transpose c2 -> cT [1, 64],
            # cast to fp16, outer-product with a ones column to get
            # cb[128, 64] (cb[p, b] = c[b]). Keeps every engine access at
            # partition base 0 (arbitrary partition bases fail birverifier).
            ctp = psX.tile([128, B], f32, name="ctp", tag="psx")
            nc.tensor.transpose(ctp[0:1, 0:B], c2[:], ident[:B, :B])
            cT16 = smallp.tile([1, B], f16)
            nc.scalar.copy(cT16[:], ctp[0:1, 0:B])
            ones16 = constp.tile([1, 128], f16)
            nc.vector.memset(ones16[:], 1.0)
            cbp = psX.tile([128, B], f32, name="cbp", tag="psx")
            nc.tensor.matmul(cbp[:], ones16[:], cT16[:], start=True, stop=True,
                             skip_group_check=True)
            cb = smallp.tile([128, B], f32)
            nc.scalar.copy(cb[:], cbp[:])

            # Main stream: one [128, 4096] fp16 tile per b-pair; 8-matmul
            # PSUM chain over hb produces ps[b', b2*256+j] = u[b'].enc[l0+j,
            # 2bp+b2]; only rows 2bp (first half) / 2bp+1 (second half) are
            # real energies. Extract via bulk PSUM->SBUF copy + four PE
            # transposes + single-column copies into eT tiles laid out
            # [l(128 part), b(64 free)] so the softmax needs no final
            # transpose.
            eT = [constp.tile([128, B], f32, name=f"eT{h}", tag=f"eT{h}")
                  for h in range(2)]

            def emit_transposes(s2, bpA, bpB):
                # 4 [128,128] PE transposes cover both stacked chains; copy
                # out the 2 useful columns each.
                for q in range(4):
                    half, b2 = q % 2, q // 2
                    pt = psX.tile([128, 128], f32, name="pt", tag="psx")
                    nc.tensor.transpose(
                        pt[:], s2[:, q * 128 : (q + 1) * 128], ident[:]
                    )
                    bA, bB = 2 * bpA + b2, 2 * bpB + b2
                    nc.scalar.copy(eT[half][:, bA : bA + 1], pt[:, bA : bA + 1])
                    nc.vector.tensor_copy(
                        eT[half][:, bB : bB + 1], pt[:, B + bB : B + bB + 1]
                    )

            # Software pipeline. Per group g: chains (PE), then the PSUM->
            # SBUF bulk copies for g IMMEDIATELY (so their semaphore waits
            # only cover g's chains — issuing them later coarsens the wait
            # to include the next group's matmuls), then group g-1's
            # transposes, which find s2(g-1) long since written.
            pend = None
            for g in range(NBP // 2):
                ts, chains = [], []
                for j in range(2):
                    bp = 2 * g + j
                    t = encp.tile([128, 4096], f16)
                    enc_dma = nc.sync.dma_start(t[:], enc_in[bp])
                    if bp < 4:
                        for w in pre_dmas:
                            add_dep_helper(
                                enc_dma.ins, w, sync=False,
                                reason="precompute DMAs drain before enc stream",
                            )
                    ts.append(t)
                    chains.append(psS.tile([B, 512], f32, name="ps", tag="ps"))
                for hb in range(8):
                    for j in range(2):
                        nc.tensor.matmul(
                            chains[j][:],
                            uT[hb][:],
                            ts[j][:, hb * 512 : (hb + 1) * 512],
                            start=(hb == 0),
                            stop=(hb == 7),
                            skip_group_check=True,
                        )
                # Stack both chains into s2 (chain A rows 0..63, B rows
                # 64..127); these run on ACT/DVE concurrent with the next
                # group's chains on PE.
                s2 = smallp.tile([128, 512], f32, name="s2", tag="s2", bufs=3)
                nc.scalar.copy(s2[0:B, :], chains[0][:])
                nc.vector.tensor_copy(s2[B:128, :], chains[1][:])
                if pend is not None:
                    emit_transposes(*pend)
                pend = (s2, 2 * g, 2 * g + 1)
            emit_transposes(*pend)

            for half in range(2):
                eTc = smallp.tile([128, B], f32)
                nc.vector.tensor_add(eTc[:], eT[half][:], cb[:])
                nm = smallp.tile([128, 1], f32)
                nc.vector.tensor_reduce(
                    nm[:],
                    eTc[:],
                    axis=mybir.AxisListType.X,
                    op=mybir.AluOpType.max,
                    negate=True,
                )
                ex = smallp.tile([128, B], f32)
                ssum = smallp.tile([128, 1], f32)
                nc.scalar.activation(
                    ex[:],
                    eTc[:],
                    mybir.ActivationFunctionType.Exp,
                    bias=nm[:, 0:1],
                    scale=1.0,
                    accum_out=ssum[:],
                )
                rec = smallp.tile([128, 1], f32)
                nc.vector.reciprocal(rec[:], ssum[:])
                attn_sb = smallp.tile([128, B], f32)
                nc.vector.tensor_scalar_mul(attn_sb[:], ex[:], rec[:, 0:1])
                nc.sync.dma_start(out_t[half * 128 : (half + 1) * 128, :], attn_sb[:])

    nc.finalize()
    return nc


def _get_program():
    global _PROGRAM
    if _PROGRAM is None:
        _PROGRAM = _build_program()
    return _PROGRAM


def _prep_inputs(inputs):
    """Build the 8 per-core input maps (all fp16 except nothing)."""
    f16 = np.float16
    hidden = np.asarray(inputs["hidden"], dtype=np.float32)
    enc = np.asarray(inputs["encoder_outputs"], dtype=np.float32)
    W = np.asarray(inputs["W"], dtype=np.float32)
    b = np.asarray(inputs["b"], dtype=np.float32)

    # enc[l, b, h] -> per core: tile[bp, p, hb*512 + b2*256 + j]
    #   = enc[core*256 + j, 2bp + b2, hb*128 + p]
    enc16 = np.ascontiguousarray(enc).astype(f16)
    E = enc16.reshape(N_CORES, L_SHARD, NBP, 2, 8, 128)  # [core, j, bp, b2, hb, p]
    P = np.ascontiguousarray(E.transpose(0, 2, 5, 4, 3, 1))  # [core, bp, p, hb, b2, j]
    P = P.reshape(N_CORES, NBP, 128, 4096)

    ht = np.ascontiguousarray(
        hidden[0].astype(f16).reshape(B, 8, 128).transpose(2, 1, 0).reshape(128, 512)
    )
    w = np.ascontiguousarray(W.astype(f16).reshape(8, 128, H))
    bv = np.ascontiguousarray(b.astype(f16).reshape(8, 128).T)

    return [
        {"enc": P[k], "ht": ht, "w": w, "bv": bv} for k in range(N_CORES)
    ]


def kernel(**inputs) -> np.ndarray:
    from concourse.bass_utils import run_bass_kernel_spmd

    nc = _get_program()
    in_maps = _prep_inputs(inputs)
    res = run_bass_kernel_spmd(nc, in_maps, list(range(N_CORES)))

    outs = []
    for k in range(N_CORES):
        a = np.asarray(res.results[k]["attn"])  # [L_SHARD, B]
        outs.append(a.T)                        # [B, L_SHARD]
    out = np.concatenate(outs, axis=1)[:, None, :].astype(np.float32)
    return out
